# revision 9
# baseline (speedup 1.0000x reference)
"""Trainium2 Bass kernel for nn_ABC_2D_Large (hash-indexed im2col conv + GEMM).

Math: out[o, b, s] = sum_{c,k} W[o, c*25+k] * keep[b,c,s,k] * x[b, c, J[c,s,k]]
where J = conv_hash with per-(b,c) block offsets removed (the generator makes
indices batch-invariant: conv_hash[b] = J + c*4096 + b*C*4096).

Distribution: spatial shard — NeuronCore cid owns s in [cid*512, (cid+1)*512).
Within a core, the 8 GPSIMD Q7 sub-cores each own a 64-wide s chunk; the 16
partitions of a Q7 sub-core hold the 16 batches, which share gather indices
(the batch-invariance makes ap_gather's shared-per-core index stream exact).

Device pipeline per NeuronCore:
  phase 1 (per 4-channel window w of 16):
    ap_gather: G[(g,b), (sl,rp)] = x[b, 4w+rp//25, J]   (6400 idx/substream)
    PE transpose (via identity matmul) 128x100 -> psum [rp, (g,b)]
    DVE: rhs = psum * keepmask  (bf16)  -> staged to HBM
  phase 2: GEMM out[o, cols] = sum_w W_w.T @ rhs_w with PSUM accumulation.
"""

import numpy as np
import ml_dtypes

B, C, H, W_ = 16, 64, 64, 64
HW = H * W_          # 4096 table entries per (b, c) plane
S = 64 * 64          # spatial outputs per (b, c)
KL = 25
OUT = 256
NCORE = 8
SPC = S // NCORE     # 512 spatial per NeuronCore
G8 = 8               # Q7 sub-cores
SLG = SPC // G8      # 64 spatial per Q7 sub-core
CW = 4               # channels per window
NWIN = C // CW       # 16
RPW = CW * KL        # 100 rows (c_loc*25+k) per window
NIDX = SLG * RPW     # 6400 gather idx per sub-core per window
TABE = CW * HW       # 16384 table elems per partition per window

_prog_cache = {}


def _build_program():
    import concourse.bass as bass
    import concourse.mybir as mybir
    import concourse.tile as tile
    from concourse import bacc
    from concourse._compat import get_trn_type

    f32 = mybir.dt.float32
    bf16 = mybir.dt.bfloat16
    i16 = mybir.dt.int16

    nc = bacc.Bacc(get_trn_type() or "TRN2", debug=False)
    tab_d = nc.dram_tensor("tab", [NWIN, B, TABE], f32, kind="ExternalInput")
    idx_d = nc.dram_tensor("idx", [NWIN, 128, NIDX // 16], i16,
                           kind="ExternalInput")
    msk_d = nc.dram_tensor("msk", [NWIN, RPW, SLG * 128], mybir.dt.uint8,
                           kind="ExternalInput")
    wt_d = nc.dram_tensor("wt", [RPW, NWIN * OUT], bf16, kind="ExternalInput")
    id_d = nc.dram_tensor("ident", [128, 128], f32, kind="ExternalInput")
    out_d = nc.dram_tensor("out", [2, 128, SLG * 128], f32,
                           kind="ExternalOutput")

    with tile.TileContext(nc) as tc:
        with (
            tc.tile_pool(name="tabp", bufs=2) as tabp,
            tc.tile_pool(name="gp", bufs=1) as gp,
            tc.tile_pool(name="idxp", bufs=2) as idxp,
            tc.tile_pool(name="mskp", bufs=1) as mskp,
            tc.tile_pool(name="rhsp", bufs=2) as rhsp,
            tc.tile_pool(name="wp", bufs=1) as wp,
            tc.tile_pool(name="outp", bufs=1) as outp,
            tc.tile_pool(name="ptp", bufs=2, space="PSUM") as ptp,
            tc.tile_pool(name="psp", bufs=1, space="PSUM") as psp,
            tc.tile_pool(name="dramp", bufs=1, space="DRAM") as dramp,
        ):
            ident = wp.tile([128, 128], f32)
            nc.sync.dma_start(ident[:], id_d[:])
            wt_all = wp.tile([RPW, NWIN * OUT], bf16)
            nc.sync.dma_start(wt_all[:], wt_d[:])

            rhs_hbm = dramp.tile([NWIN, RPW, SLG * 128], bf16)

            # ---- phase 1: gather + transpose + mask ----
            for w in range(NWIN):
                tab_t = tabp.tile([128, TABE], f32)
                tsrc = bass.AP(tensor=tab_d[w].tensor,
                               offset=tab_d[w].offset,
                               ap=[[0, G8], [TABE, B], [1, TABE]])
                nc.sync.dma_start(tab_t[:], tsrc)
                idx_t = idxp.tile([128, NIDX // 16], i16)
                nc.sync.dma_start(idx_t[:], idx_d[w])
                msk_u = mskp.tile([RPW, SLG * 128], mybir.dt.uint8, tag="msku")
                nc.scalar.dma_start(msk_u[:], msk_d[w])

                g_t = gp.tile([128, NIDX], f32)
                nc.gpsimd.ap_gather(
                    g_t[:].rearrange("p (n d) -> p n d", d=1),
                    tab_t[:].rearrange("p (n d) -> p n d", d=1),
                    idx_t[:],
                    channels=128,
                    num_elems=TABE,
                    d=1,
                    num_idxs=NIDX,
                )

                rhs_st = rhsp.tile([RPW, SLG * 128], bf16)
                for s4 in range(SLG // 4):
                    pt = ptp.tile([RPW, 512], f32)
                    for q in range(4):
                        sl = s4 * 4 + q
                        nc.tensor.transpose(
                            pt[:, q * 128:(q + 1) * 128],
                            g_t[:, sl * RPW:(sl + 1) * RPW],
                            ident[:],
                        )
                    cols = slice(s4 * 512, (s4 + 1) * 512)
                    nc.vector.tensor_tensor(
                        rhs_st[:, cols], pt[:], msk_u[:, cols],
                        mybir.AluOpType.mult,
                    )
                nc.sync.dma_start(rhs_hbm[w], rhs_st[:])

            # ---- phase 2: GEMM with PSUM accumulation over windows ----
            for sq in range(4):
                for nch in range(4):
                    cbase = sq * 2048 + nch * 512
                    ps = [psp.tile([128, 512], f32, name=f"ps{_m}", tag=f"ps{_m}")
                          for _m in range(2)]
                    for kt in range(NWIN):
                        rt = idxp.tile([RPW, 512], bf16, tag="rt")
                        nc.sync.dma_start(
                            rt[:], rhs_hbm[kt][:, cbase:cbase + 512])
                        for m in range(2):
                            nc.tensor.matmul(
                                ps[m][:],
                                wt_all[:, kt * OUT + m * 128:
                                       kt * OUT + (m + 1) * 128],
                                rt[:],
                                start=(kt == 0),
                                stop=(kt == NWIN - 1),
                            )
                    for m in range(2):
                        ot = outp.tile([128, 512], f32)
                        nc.vector.tensor_copy(ot[:], ps[m][:])
                        nc.sync.dma_start(
                            out_d[m][:, cbase:cbase + 512], ot[:])
    nc.compile()
    return nc


def _host_prep(x, conv_hash, zerofy, weights):
    """Verify generator structure; build per-core device tensors."""
    ch = np.asarray(conv_hash)
    il0 = ch[0].astype(np.int64)                       # [C, 64, 64, KL]
    boff = (np.arange(B, dtype=np.int64) * (C * HW))
    if not np.array_equal(
            ch.astype(np.int64),
            il0[None] + boff[:, None, None, None, None]):
        raise RuntimeError(
            "conv_hash lacks the batch-invariant structure this kernel "
            "is specialized for")
    IL = il0.reshape(C, S, KL) - np.arange(C, dtype=np.int64)[:, None, None] * HW
    if IL.min() < 0 or IL.max() >= HW:
        raise RuntimeError("conv_hash channel offsets unexpected")
    IL = IL.astype(np.int32)                           # [C, S, KL] in [0, HW)

    rp = np.arange(RPW)
    cl = rp // KL                                      # [RPW] in [0, CW)
    kk = rp % KL

    # E[w, s, rp] = cl*HW + IL[4w+cl, s, kk]   (int16-safe: < 16384)
    cidx = (CW * np.arange(NWIN)[:, None, None] + cl[None, None, :])
    E = IL[cidx, np.arange(S)[None, :, None], kk[None, None, :]] \
        + cl[None, None, :] * HW
    E = E.astype(np.int16)                             # [NWIN, S, RPW]

    # tab[w, g*16+b, cl*HW+j] = x[b, CW*w+cl, j]
    xt = np.ascontiguousarray(
        np.asarray(x, dtype=np.float32).reshape(B, NWIN, TABE)
        .transpose(1, 0, 2))                           # [NWIN, B, TABE]
    tab = xt

    keep = (~np.asarray(zerofy)).reshape(B, C, S, KL)
    # A[c, k, s, b]
    A = np.ascontiguousarray(keep.transpose(1, 3, 2, 0)).astype(np.uint8)

    wt = np.ascontiguousarray(
        np.asarray(weights, dtype=np.float32).T.reshape(NWIN, RPW, OUT)
        .transpose(1, 0, 2).reshape(RPW, NWIN * OUT)).astype(
        ml_dtypes.bfloat16)

    ident = np.eye(128, dtype=np.float32)

    in_maps = []
    for cid in range(NCORE):
        sly = slice(cid * SPC, (cid + 1) * SPC)
        # idx streams: Ec[w, g, sl, rp] -> wrap per sub-core
        Ec = E[:, sly, :].reshape(NWIN, G8, SLG, RPW)
        idx = np.ascontiguousarray(
            Ec.reshape(NWIN, G8, NIDX // 16, 16)
            .transpose(0, 1, 3, 2)                     # [w, g, 16, NIDX/16]
            .reshape(NWIN, 128, NIDX // 16))
        # msk[w, rp, (sl, g, b)] = keep[b, CW*w+cl, cid*512+g*64+sl, kk]
        Ac = A[:, :, sly, :].reshape(C, KL, G8, SLG, B)
        # -> [w, rp, sl, g, b]
        M = Ac[cidx[:, 0, :], kk[None, :]]             # [NWIN, RPW, G8, SLG, B]
        M = np.ascontiguousarray(
            M.transpose(0, 1, 3, 2, 4).reshape(NWIN, RPW, SLG * 128))
        in_maps.append({
            "tab": tab, "idx": idx, "msk": M, "wt": wt, "ident": ident,
        })
    return in_maps


def _reassemble(results):
    # per core: out[m, ol, sl*128 + g*16 + b] ; s = cid*512 + g*64 + sl
    out = np.empty((B, OUT, S), dtype=np.float32)
    for cid in range(NCORE):
        rc = np.asarray(results[cid]["out"], dtype=np.float32)
        rc = rc.reshape(2, 128, SLG, G8, B)            # [m, ol, sl, g, b]
        rc = rc.transpose(4, 0, 1, 3, 2)               # [b, m, ol, g, sl]
        out[:, :, cid * SPC:(cid + 1) * SPC] = rc.reshape(B, OUT, SPC)
    return out.reshape(B, OUT, 64, 64)


def kernel(x, conv_hash, zerofy, weights):
    from concourse.bass_utils import run_bass_kernel_spmd

    if "nc" not in _prog_cache:
        _prog_cache["nc"] = _build_program()
    nc = _prog_cache["nc"]
    in_maps = _host_prep(x, conv_hash, zerofy, weights)
    res = run_bass_kernel_spmd(nc, in_maps, core_ids=list(range(NCORE)))
    return _reassemble(res.results)
